# revision 7
# baseline (speedup 1.0000x reference)
"""Trainium2 Bass kernel for 3-layer GAT + BN/ELU (nn_GAT_BN_60859686584881).

Strategy: dst-sorted edges, node-blocks of 128 per core (graph-parallel over 8
cores). Per 128-edge chunk: indirect-DMA gather of table rows [h|alpha_src],
selection-matrix build via is_equal vs iota, attention softmax without max
subtraction (2-pass: accumulate exp-weighted messages + exp sums via PE
matmuls into PSUM, normalize at block end). Layer tables exchanged with
AllGather collectives; alpha_dst kept core-local in SBUF.
"""
import sys
sys.path.insert(0, '/opt/trn_rl_repo')
import numpy as np

import concourse.bacc as bacc
import concourse.bass as bass
import concourse.tile as tile
import concourse.mybir as mybir
from concourse import bass_utils
from concourse.masks import make_identity

N = 50000
E = 800000
F_IN, HID, H, LBL = 512, 16, 8, 40
HC = HID * H  # 128
BN_EPS = 1e-5
P = 128
NCORE = 8
NBLK = 49                      # blocks per core
ROWS_PER_CORE = NBLK * P       # 6272
NTAB = NCORE * ROWS_PER_CORE   # 50176 gathered table rows
NZPAD = 128                    # zero rows appended for padded gather slots
W1T = HC + H                   # 136: [h | alpha_src] layer0/1 table width
W2T = LBL + 1                  # 41:  layer2 table width

f32 = mybir.dt.float32
i32 = mybir.dt.int32

_CACHE = {}


def _preprocess(edge_index):
    src = edge_index[0].astype(np.int64)
    dst = edge_index[1].astype(np.int64)
    loops = np.arange(N, dtype=np.int64)
    src = np.concatenate([src, loops])
    dst = np.concatenate([dst, loops])
    order = np.argsort(dst, kind='stable')
    src, dst = src[order], dst[order]

    nblk_total = NCORE * NBLK  # 392 block slots; real blocks 0..390
    counts = np.bincount((dst // P).astype(np.int64), minlength=nblk_total)
    ptr = np.concatenate([[0], np.cumsum(counts)])
    nch = int(np.ceil(counts.max() / P))  # uniform chunks per block

    # per-core [128, NBLK*nch] arrays
    offs = np.empty((NCORE, P, NBLK * nch), np.int32)
    dloc = np.empty((NCORE, P, NBLK * nch), np.float32)
    # padded slots: spread indices into the zero-row region, -1 dst_local
    spread = (NTAB + (np.arange(P) % NZPAD)).astype(np.int32)
    for c in range(NCORE):
        for b in range(NBLK):
            g = c * NBLK + b
            e0, e1 = ptr[g], ptr[g + 1]
            es = src[e0:e1]
            ed = dst[e0:e1] - g * P
            ne = e1 - e0
            col = np.tile(spread[:, None], (1, nch))
            dcol = np.full((P, nch), -1.0, np.float32)
            if ne:
                flat_i = np.full(nch * P, -1, np.int64)
                flat_i[:ne] = es
                flat_d = np.full(nch * P, -1.0, np.float32)
                flat_d[:ne] = ed
                ii = flat_i.reshape(nch, P).T  # [P, nch]
                dd = flat_d.reshape(nch, P).T
                m = ii >= 0
                col[m] = ii[m]
                dcol[m] = dd[m]
            offs[c, :, b * nch:(b + 1) * nch] = col
            dloc[c, :, b * nch:(b + 1) * nch] = dcol
    return offs, dloc, nch


def _build_program(nch):
    nc = bacc.Bacc("TRN2", target_bir_lowering=False, debug=False,
                   enable_asserts=False, num_devices=NCORE)
    NCH_T = NBLK * nch

    xT_t = nc.dram_tensor("xT", [F_IN, ROWS_PER_CORE], f32, kind="ExternalInput")
    offs_t = nc.dram_tensor("offs", [P, NCH_T], i32, kind="ExternalInput")
    dloc_t = nc.dram_tensor("dloc", [P, NCH_T], f32, kind="ExternalInput")
    w0_t = nc.dram_tensor("w0ext", [F_IN, W1T + H], f32, kind="ExternalInput")
    w1_t = nc.dram_tensor("w1ext", [HC, W1T + H], f32, kind="ExternalInput")
    w2_t = nc.dram_tensor("w2ext", [HC, W2T + 1], f32, kind="ExternalInput")
    bn0_t = nc.dram_tensor("bn0", [2 * P, HC], f32, kind="ExternalInput")  # scale, shift row-replicated
    bn1_t = nc.dram_tensor("bn1", [2 * P, HC], f32, kind="ExternalInput")
    b2_t = nc.dram_tensor("b2", [P, LBL], f32, kind="ExternalInput")
    out_t = nc.dram_tensor("out", [ROWS_PER_CORE, LBL], f32, kind="ExternalOutput")
    dbg0_t = nc.dram_tensor("dbg0", [ROWS_PER_CORE, W1T], f32, kind="ExternalOutput")
    dbg1_t = nc.dram_tensor("dbg1", [ROWS_PER_CORE, W1T], f32, kind="ExternalOutput")

    # internal DRAM: per-layer shard + gathered tables
    shard = [nc.dram_tensor(f"shard{l}", [ROWS_PER_CORE, [W1T, W1T, W2T][l]], f32,
                            kind="Internal") for l in range(3)]
    tabs = [nc.dram_tensor(f"tab{l}", [NTAB + NZPAD, [W1T, W1T, W2T][l]], f32,
                           kind="Internal", addr_space="Shared") for l in range(3)]

    with tile.TileContext(nc) as tc:
        with tc.tile_pool(name="sbuf", bufs=1) as sb, \
             tc.tile_pool(name="psum", bufs=1, space="PSUM") as pp:

            ident = sb.tile([P, P], f32, name="ident")
            make_identity(nc, ident[:])
            iota_row = sb.tile([P, P], f32, name="iota_row")
            nc.gpsimd.iota(iota_row[:], pattern=[[1, P]], base=0,
                           channel_multiplier=0,
                           allow_small_or_imprecise_dtypes=True)
            offs_sb = sb.tile([P, NCH_T], i32, name="offs_sb")
            nc.sync.dma_start(offs_sb[:], offs_t[:])
            dloc_sb = sb.tile([P, NCH_T], f32, name="dloc_sb")
            nc.sync.dma_start(dloc_sb[:], dloc_t[:])
            w1_sb = sb.tile([HC, W1T + H], f32, name="w1_sb")
            nc.sync.dma_start(w1_sb[:], w1_t[:])
            w2_sb = sb.tile([HC, W2T + 1], f32, name="w2_sb")
            nc.sync.dma_start(w2_sb[:], w2_t[:])
            bn_sb = [sb.tile([P, 2 * HC], f32, name=f"bn_sb{l}") for l in range(2)]
            nc.sync.dma_start(bn_sb[0][:, :HC], bn0_t[0:P, :])
            nc.sync.dma_start(bn_sb[0][:, HC:], bn0_t[P:2 * P, :])
            nc.sync.dma_start(bn_sb[1][:, :HC], bn1_t[0:P, :])
            nc.sync.dma_start(bn_sb[1][:, HC:], bn1_t[P:2 * P, :])
            b2_sb = sb.tile([P, LBL], f32, name="b2_sb")
            nc.sync.dma_start(b2_sb[:], b2_t[:])
            # alpha_dst for own rows, per layer: [P, NBLK*H]
            ad_sb = [sb.tile([P, NBLK * (H if l < 2 else 1)], f32, name=f"ad_sb{l}")
                     for l in range(3)]
            zrow = sb.tile([P, W1T], f32, name="zrow")
            nc.vector.memset(zrow[:], 0.0)
            for l in range(3):
                wl = [W1T, W1T, W2T][l]
                nc.sync.dma_start(tabs[l][NTAB:NTAB + NZPAD, :], zrow[:, :wl])

            # ---------- layer 0 prologue: shard rows of table0 = x @ W0ext ----
            w0_sb = [sb.tile([P, W1T + H], f32, name=f"w0_sb{k}") for k in range(4)]
            for k in range(4):
                nc.sync.dma_start(w0_sb[k][:], w0_t[k * P:(k + 1) * P, :])
            xT_sb = [sb.tile([P, ROWS_PER_CORE], f32, name=f"xT_sb{k}") for k in range(4)]
            for k in range(4):
                nc.sync.dma_start(xT_sb[k][:], xT_t[k * P:(k + 1) * P, :])
            for b in range(NBLK):
                ps = pp.tile([P, W1T + H], f32, name="ps_pro", tag="misc_ps", bufs=1)
                for k in range(4):
                    nc.tensor.matmul(
                        out=ps[:],
                        lhsT=xT_sb[k][:, b * P:(b + 1) * P],
                        rhs=w0_sb[k][:],
                        start=(k == 0), stop=(k == 3))
                row_sb = sb.tile([P, W1T + H], f32, name="row_pro", tag="row_pro", bufs=2)
                nc.vector.tensor_copy(row_sb[:], ps[:])
                nc.sync.dma_start(shard[0][b * P:(b + 1) * P, :], row_sb[:, :W1T])
                nc.sync.dma_start(dbg0_t[b * P:(b + 1) * P, :], row_sb[:, :W1T])
                nc.vector.tensor_copy(ad_sb[0][:, b * H:(b + 1) * H],
                                      row_sb[:, W1T:W1T + H])
            nc.gpsimd.collective_compute(
                "AllGather", mybir.AluOpType.bypass,
                replica_groups=[list(range(NCORE))],
                ins=[shard[0][:]], outs=[tabs[0][0:NTAB, :]])

            # ---------- edge phases ----------
            for l in range(3):
                wl, nh, ch = ([W1T, W1T, W2T][l], [H, H, 1][l], [HID, HID, LBL][l])
                hw = nh * ch  # 128 / 128 / 40
                for b in range(NBLK):
                    ps_out = pp.tile([P, hw], f32, name=f"ps_out{l}", tag="ps_out", bufs=2)
                    ps_s = pp.tile([P, nh], f32, name=f"ps_s{l}", tag="ps_s", bufs=1)
                    for c in range(nch):
                        cc = b * nch + c
                        g = sb.tile([P, wl], f32, name=f"g{l}", tag="g", bufs=6)
                        nc.gpsimd.indirect_dma_start(
                            out=g[:], out_offset=None, in_=tabs[l][:],
                            in_offset=bass.IndirectOffsetOnAxis(
                                ap=offs_sb[:, cc:cc + 1], axis=0))
                        selT = sb.tile([P, P], f32, name=f"selT{l}", tag="selT", bufs=3)
                        nc.vector.tensor_tensor(
                            out=selT[:],
                            in0=dloc_sb[:, cc:cc + 1].to_broadcast([P, P]),
                            in1=iota_row[:], op=mybir.AluOpType.is_equal)
                        sel_ps = pp.tile([P, P], f32, name=f"sel_ps{l}", tag="sel_ps", bufs=2)
                        nc.tensor.transpose(out=sel_ps[:], in_=selT[:], identity=ident[:])
                        sel_sb = sb.tile([P, P], f32, name=f"sel_sb{l}", tag="sel_sb", bufs=3)
                        nc.vector.tensor_copy(sel_sb[:], sel_ps[:])
                        ad_ps = pp.tile([P, nh], f32, name=f"ad_ps{l}", tag="ad_ps", bufs=1)
                        nc.tensor.matmul(out=ad_ps[:], lhsT=sel_sb[:],
                                         rhs=ad_sb[l][:, b * nh:(b + 1) * nh],
                                         start=True, stop=True)
                        u = sb.tile([P, nh], f32, name=f"u{l}", tag="u", bufs=3)
                        nc.vector.tensor_tensor(out=u[:], in0=g[:, hw:hw + nh],
                                                in1=ad_ps[:], op=mybir.AluOpType.add)
                        un = sb.tile([P, nh], f32, name=f"un{l}", tag="un", bufs=3)
                        nc.vector.tensor_scalar_mul(un[:], u[:], 0.2)
                        nc.vector.tensor_tensor(out=u[:], in0=u[:], in1=un[:],
                                                op=mybir.AluOpType.max)
                        ee = sb.tile([P, nh], f32, name=f"ee{l}", tag="ee", bufs=3)
                        nc.scalar.activation(ee[:], u[:],
                                             mybir.ActivationFunctionType.Exp)
                        m = sb.tile([P, hw], f32, name=f"m{l}", tag="m", bufs=3)
                        nc.vector.tensor_tensor(
                            out=m[:].rearrange("p (h c) -> p h c", h=nh),
                            in0=g[:, :hw].rearrange("p (h c) -> p h c", h=nh),
                            in1=ee[:].unsqueeze(-1).to_broadcast([P, nh, ch]),
                            op=mybir.AluOpType.mult)
                        nc.tensor.matmul(out=ps_out[:], lhsT=selT[:], rhs=m[:],
                                         start=(c == 0), stop=(c == nch - 1),
                                         skip_group_check=True)
                        nc.tensor.matmul(out=ps_s[:], lhsT=selT[:], rhs=ee[:],
                                         start=(c == 0), stop=(c == nch - 1),
                                         skip_group_check=True)
                    # ---- block finalize ----
                    rs = sb.tile([P, nh], f32, name=f"rs{l}", tag="rs", bufs=2)
                    nc.vector.reciprocal(rs[:], ps_s[:])
                    ob = sb.tile([P, hw], f32, name=f"ob{l}", tag="ob", bufs=2)
                    nc.vector.tensor_tensor(
                        out=ob[:].rearrange("p (h c) -> p h c", h=nh),
                        in0=ps_out[:].rearrange("p (h c) -> p h c", h=nh),
                        in1=rs[:].unsqueeze(-1).to_broadcast([P, nh, ch]),
                        op=mybir.AluOpType.mult)
                    if l < 2:
                        # BN (scale/shift per column) + ELU
                        nc.vector.tensor_tensor(
                            out=ob[:], in0=ob[:],
                            in1=bn_sb[l][:, :HC],
                            op=mybir.AluOpType.mult)
                        nc.vector.tensor_tensor(
                            out=ob[:], in0=ob[:],
                            in1=bn_sb[l][:, HC:],
                            op=mybir.AluOpType.add)
                        tneg = sb.tile([P, hw], f32, name=f"tneg{l}", tag="tneg", bufs=2)
                        nc.vector.tensor_scalar_min(tneg[:], ob[:], 0.0)
                        nc.scalar.activation(tneg[:], tneg[:],
                                             mybir.ActivationFunctionType.Exp)
                        # tneg = 0.2*exp(min(y,0)) - 0.2
                        nc.vector.tensor_scalar(
                            out=tneg[:], in0=tneg[:], scalar1=0.2, scalar2=-0.2,
                            op0=mybir.AluOpType.mult, op1=mybir.AluOpType.add)
                        nc.vector.tensor_scalar_max(ob[:], ob[:], 0.0)
                        nc.vector.tensor_tensor(out=ob[:], in0=ob[:], in1=tneg[:],
                                                op=mybir.AluOpType.add)
                        # next layer rows: eluT @ W_{l+1}ext
                        eT_ps = pp.tile([P, P], f32, name=f"eT_ps{l}", tag="misc_ps", bufs=1)
                        nc.tensor.transpose(out=eT_ps[:], in_=ob[:], identity=ident[:])
                        eT_sb = sb.tile([P, P], f32, name=f"eT_sb{l}", tag="eT_sb", bufs=2)
                        nc.vector.tensor_copy(eT_sb[:], eT_ps[:])
                        wnext = w1_sb if l == 0 else w2_sb
                        wn = W1T + H if l == 0 else W2T + 1
                        nhn = H if l == 0 else 1
                        row_ps = pp.tile([P, wn], f32, name=f"row_ps{l}", tag="misc_ps", bufs=1)
                        nc.tensor.matmul(out=row_ps[:], lhsT=eT_sb[:],
                                         rhs=wnext[:, :wn], start=True, stop=True)
                        row_sb2 = sb.tile([P, wn], f32, name=f"row_sb2{l}", tag="row_sb2", bufs=2)
                        nc.vector.tensor_copy(row_sb2[:], row_ps[:])
                        nc.sync.dma_start(shard[l + 1][b * P:(b + 1) * P, :],
                                          row_sb2[:, :wn - nhn])
                        if l == 0:
                            nc.sync.dma_start(dbg1_t[b * P:(b + 1) * P, :],
                                              row_sb2[:, :W1T])
                        nc.vector.tensor_copy(
                            ad_sb[l + 1][:, b * nhn:(b + 1) * nhn],
                            row_sb2[:, wn - nhn:wn])
                    else:
                        nc.vector.tensor_tensor(
                            out=ob[:], in0=ob[:],
                            in1=b2_sb[:],
                            op=mybir.AluOpType.add)
                        nc.sync.dma_start(out_t[b * P:(b + 1) * P, :], ob[:])
                if l < 2:
                    nc.gpsimd.collective_compute(
                        "AllGather", mybir.AluOpType.bypass,
                        replica_groups=[list(range(NCORE))],
                        ins=[shard[l + 1][:]], outs=[tabs[l + 1][0:NTAB, :]])
    nc.compile()
    return nc


def _host_inputs(inputs, offs, dloc):
    x = np.asarray(inputs['x'], np.float32)
    xpad = np.zeros((NCORE * ROWS_PER_CORE, F_IN), np.float32)
    xpad[:N] = x

    def bd(a):  # [H, C] -> block-diag [H*C, H]
        hh, cc = a.shape
        m = np.zeros((hh * cc, hh), np.float32)
        for h in range(hh):
            m[h * cc:(h + 1) * cc, h] = a[h]
        return m

    W0, W1, W2 = [np.asarray(inputs[k], np.float32) for k in ('W0', 'W1', 'W2')]
    w0ext = np.concatenate([W0, W0 @ bd(np.asarray(inputs['as0'])),
                            W0 @ bd(np.asarray(inputs['ad0']))], axis=1)
    w1ext = np.concatenate([W1, W1 @ bd(np.asarray(inputs['as1'])),
                            W1 @ bd(np.asarray(inputs['ad1']))], axis=1)
    w2ext = np.concatenate([W2, W2 @ bd(np.asarray(inputs['as2'])),
                            W2 @ bd(np.asarray(inputs['ad2']))], axis=1)

    def bnfold(g, be, rm, rv, b):
        sc = np.asarray(g) / np.sqrt(np.asarray(rv) + BN_EPS)
        sh = (np.asarray(b) - np.asarray(rm)) * sc + np.asarray(be)
        return np.concatenate([np.tile(sc[None, :], (P, 1)), np.tile(sh[None, :], (P, 1))]).astype(np.float32)

    bn0 = bnfold(inputs['g0'], inputs['be0'], inputs['rm0'], inputs['rv0'], inputs['b0'])
    bn1 = bnfold(inputs['g1'], inputs['be1'], inputs['rm1'], inputs['rv1'], inputs['b1'])
    b2 = np.tile(np.asarray(inputs['b2'], np.float32)[None, :], (P, 1))

    in_maps = []
    for c in range(NCORE):
        xT = xpad[c * ROWS_PER_CORE:(c + 1) * ROWS_PER_CORE].T.copy()
        in_maps.append({
            'xT': xT, 'offs': offs[c], 'dloc': dloc[c],
            'w0ext': w0ext, 'w1ext': w1ext, 'w2ext': w2ext,
            'bn0': bn0, 'bn1': bn1, 'b2': b2,
        })
    return in_maps


def kernel(**inputs):
    edge_index = np.asarray(inputs['edge_index'])
    offs, dloc, nch = _preprocess(edge_index)
    if nch not in _CACHE:
        _CACHE[nch] = _build_program(nch)
    nc = _CACHE[nch]
    in_maps = _host_inputs(inputs, offs, dloc)
    res = bass_utils.run_bass_kernel_spmd(nc, in_maps, core_ids=list(range(NCORE)))
    out = np.concatenate([res.results[c]['out'] for c in range(NCORE)], axis=0)
    return out[:N].astype(np.float32)


# revision 10
# speedup vs baseline: 1.4351x; 1.4351x over previous
"""Trainium2 Bass kernel for 3-layer GAT + BN/ELU (nn_GAT_BN_60859686584881).

Strategy: dst-sorted edges, node-blocks of 128 per core (graph-parallel over 8
cores). Per 128-edge chunk: indirect-DMA gather of table rows [h|alpha_src],
selection-matrix build via is_equal vs iota, attention softmax without max
subtraction (2-pass: accumulate exp-weighted messages + exp sums via PE
matmuls into PSUM, normalize at block end). Layer tables exchanged with
AllGather collectives; alpha_dst kept core-local in SBUF.
"""
import sys
sys.path.insert(0, '/opt/trn_rl_repo')
import numpy as np

import concourse.bacc as bacc
import concourse.bass as bass
import concourse.tile as tile
import concourse.mybir as mybir
from concourse import bass_utils
from concourse.masks import make_identity

N = 50000
E = 800000
F_IN, HID, H, LBL = 512, 16, 8, 40
HC = HID * H  # 128
BN_EPS = 1e-5
P = 128
NCORE = 8
NBLK = 49                      # blocks per core
ROWS_PER_CORE = NBLK * P       # 6272
NTAB = NCORE * ROWS_PER_CORE   # 50176 gathered table rows
NZPAD = 128                    # zero rows appended for padded gather slots
W1T = HC + H                   # 136: [h | alpha_src] layer0/1 table width
W2T = LBL + 1                  # 41:  layer2 table width

f32 = mybir.dt.float32
i32 = mybir.dt.int32

_CACHE = {}


def _preprocess(edge_index):
    src = edge_index[0].astype(np.int64)
    dst = edge_index[1].astype(np.int64)
    loops = np.arange(N, dtype=np.int64)
    src = np.concatenate([src, loops])
    dst = np.concatenate([dst, loops])
    order = np.argsort(dst, kind='stable')
    src, dst = src[order], dst[order]

    nblk_total = NCORE * NBLK  # 392 block slots; real blocks 0..390
    counts = np.bincount((dst // P).astype(np.int64), minlength=nblk_total)
    ptr = np.concatenate([[0], np.cumsum(counts)])
    nch = int(np.ceil(counts.max() / P))  # uniform chunks per block

    # per-core [128, NBLK*nch] arrays
    offs = np.empty((NCORE, P, NBLK * nch), np.int32)
    dloc = np.empty((NCORE, P, NBLK * nch), np.float32)
    # padded slots: spread indices into the zero-row region, -1 dst_local
    spread = (NTAB + (np.arange(P) % NZPAD)).astype(np.int32)
    for c in range(NCORE):
        for b in range(NBLK):
            g = c * NBLK + b
            e0, e1 = ptr[g], ptr[g + 1]
            es = src[e0:e1]
            ed = dst[e0:e1] - g * P
            ne = e1 - e0
            col = np.tile(spread[:, None], (1, nch))
            dcol = np.full((P, nch), -1.0, np.float32)
            if ne:
                flat_i = np.full(nch * P, -1, np.int64)
                flat_i[:ne] = es
                flat_d = np.full(nch * P, -1.0, np.float32)
                flat_d[:ne] = ed
                ii = flat_i.reshape(nch, P).T  # [P, nch]
                dd = flat_d.reshape(nch, P).T
                m = ii >= 0
                col[m] = ii[m]
                dcol[m] = dd[m]
            offs[c, :, b * nch:(b + 1) * nch] = col
            dloc[c, :, b * nch:(b + 1) * nch] = dcol
    return offs, dloc, nch


def _build_program(nch, nlayers=3):
    nc = bacc.Bacc("TRN2", target_bir_lowering=False, debug=False,
                   enable_asserts=False, num_devices=NCORE)
    NCH_T = NBLK * nch

    xT_t = nc.dram_tensor("xT", [F_IN, ROWS_PER_CORE], f32, kind="ExternalInput")
    offs_t = nc.dram_tensor("offs", [P, NCH_T], i32, kind="ExternalInput")
    dloc_t = nc.dram_tensor("dloc", [P, NCH_T], f32, kind="ExternalInput")
    w0_t = nc.dram_tensor("w0ext", [F_IN, W1T + H], f32, kind="ExternalInput")
    w1_t = nc.dram_tensor("w1ext", [HC, W1T + H], f32, kind="ExternalInput")
    w2_t = nc.dram_tensor("w2ext", [HC, W2T + 1], f32, kind="ExternalInput")
    bn0_t = nc.dram_tensor("bn0", [2 * P, HC], f32, kind="ExternalInput")  # scale, shift row-replicated
    bn1_t = nc.dram_tensor("bn1", [2 * P, HC], f32, kind="ExternalInput")
    b2_t = nc.dram_tensor("b2", [P, LBL], f32, kind="ExternalInput")
    out_t = nc.dram_tensor("out", [ROWS_PER_CORE, LBL], f32, kind="ExternalOutput")
    dbg0_t = nc.dram_tensor("dbg0", [ROWS_PER_CORE, W1T], f32, kind="ExternalOutput")
    dbg1_t = nc.dram_tensor("dbg1", [ROWS_PER_CORE, W1T], f32, kind="ExternalOutput")

    # internal DRAM: per-layer shard + gathered tables
    shard = [nc.dram_tensor(f"shard{l}", [ROWS_PER_CORE, [W1T, W1T, W2T][l]], f32,
                            kind="Internal") for l in range(3)]
    tabs = [nc.dram_tensor(f"tab{l}", [NTAB + NZPAD, [W1T, W1T, W2T][l]], f32,
                           kind="Internal", addr_space="Shared") for l in range(3)]

    with tile.TileContext(nc) as tc:
        with tc.tile_pool(name="sbuf", bufs=1) as sb, \
             tc.tile_pool(name="psum", bufs=1, space="PSUM") as pp:

            ident = sb.tile([P, P], f32, name="ident")
            make_identity(nc, ident[:])
            iota_row = sb.tile([P, P], f32, name="iota_row")
            nc.gpsimd.iota(iota_row[:], pattern=[[1, P]], base=0,
                           channel_multiplier=0,
                           allow_small_or_imprecise_dtypes=True)
            offs_sb = sb.tile([P, NCH_T], i32, name="offs_sb")
            nc.sync.dma_start(offs_sb[:], offs_t[:])
            dloc_sb = sb.tile([P, NCH_T], f32, name="dloc_sb")
            nc.sync.dma_start(dloc_sb[:], dloc_t[:])
            w1_sb = sb.tile([HC, W1T + H], f32, name="w1_sb")
            nc.sync.dma_start(w1_sb[:], w1_t[:])
            w2_sb = sb.tile([HC, W2T + 1], f32, name="w2_sb")
            nc.sync.dma_start(w2_sb[:], w2_t[:])
            bn_sb = [sb.tile([P, 2 * HC], f32, name=f"bn_sb{l}") for l in range(2)]
            nc.sync.dma_start(bn_sb[0][:, :HC], bn0_t[0:P, :])
            nc.sync.dma_start(bn_sb[0][:, HC:], bn0_t[P:2 * P, :])
            nc.sync.dma_start(bn_sb[1][:, :HC], bn1_t[0:P, :])
            nc.sync.dma_start(bn_sb[1][:, HC:], bn1_t[P:2 * P, :])
            b2_sb = sb.tile([P, LBL], f32, name="b2_sb")
            nc.sync.dma_start(b2_sb[:], b2_t[:])
            # alpha_dst for own rows, per layer: [P, NBLK*H]
            ad_sb = [sb.tile([P, NBLK * (H if l < 2 else 1)], f32, name=f"ad_sb{l}")
                     for l in range(3)]
            zrow = sb.tile([P, W1T], f32, name="zrow")
            nc.vector.memset(zrow[:], 0.0)
            for l in range(3):
                wl = [W1T, W1T, W2T][l]
                nc.sync.dma_start(tabs[l][NTAB:NTAB + NZPAD, :], zrow[:, :wl])

            # ---------- layer 0 prologue: shard rows of table0 = x @ W0ext ----
            w0_sb = [sb.tile([P, W1T + H], f32, name=f"w0_sb{k}") for k in range(4)]
            for k in range(4):
                nc.sync.dma_start(w0_sb[k][:], w0_t[k * P:(k + 1) * P, :])
            xT_sb = [sb.tile([P, ROWS_PER_CORE], f32, name=f"xT_sb{k}") for k in range(4)]
            for k in range(4):
                nc.sync.dma_start(xT_sb[k][:], xT_t[k * P:(k + 1) * P, :])
            for b in range(NBLK):
                ps = pp.tile([P, W1T + H], f32, name="ps_pro", tag="misc_ps", bufs=1)
                for k in range(4):
                    nc.tensor.matmul(
                        out=ps[:],
                        lhsT=xT_sb[k][:, b * P:(b + 1) * P],
                        rhs=w0_sb[k][:],
                        start=(k == 0), stop=(k == 3))
                row_sb = sb.tile([P, W1T + H], f32, name="row_pro", tag="row_pro", bufs=2)
                nc.vector.tensor_copy(row_sb[:], ps[:])
                nc.sync.dma_start(shard[0][b * P:(b + 1) * P, :], row_sb[:, :W1T])
                nc.sync.dma_start(dbg0_t[b * P:(b + 1) * P, :], row_sb[:, :W1T])
                nc.vector.tensor_copy(ad_sb[0][:, b * H:(b + 1) * H],
                                      row_sb[:, W1T:W1T + H])
            nc.gpsimd.collective_compute(
                "AllGather", mybir.AluOpType.bypass,
                replica_groups=[list(range(NCORE))],
                ins=[shard[0][:]], outs=[tabs[0][0:NTAB, :]])

            # ---------- edge phases ----------
            for l in range(nlayers):
                wl, nh, ch = ([W1T, W1T, W2T][l], [H, H, 1][l], [HID, HID, LBL][l])
                hw = nh * ch  # 128 / 128 / 40
                G = 4
                for b in range(NBLK):
                    ps_out = pp.tile([P, hw], f32, name=f"ps_out{l}", tag="ps_out", bufs=2)
                    ps_s = pp.tile([P, nh], f32, name=f"ps_s{l}", tag="ps_s", bufs=1)
                    for c0 in range(0, nch, G):
                        gn = min(G, nch - c0)
                        g = sb.tile([P, G * wl], f32, name=f"g{l}", tag="g", bufs=4)
                        selT = sb.tile([P, G * P], f32, name=f"selT{l}", tag="selT", bufs=3)
                        u = sb.tile([P, G * nh], f32, name=f"u{l}", tag="u", bufs=3)
                        un = sb.tile([P, G * nh], f32, name=f"un{l}", tag="un", bufs=3)
                        ee = sb.tile([P, G * nh], f32, name=f"ee{l}", tag="ee", bufs=3)
                        m = sb.tile([P, G * hw], f32, name=f"m{l}", tag="m", bufs=3)
                        for j in range(gn):
                            cc = b * nch + c0 + j
                            nc.gpsimd.indirect_dma_start(
                                out=g[:, j * wl:(j + 1) * wl], out_offset=None,
                                in_=tabs[l][:],
                                in_offset=bass.IndirectOffsetOnAxis(
                                    ap=offs_sb[:, cc:cc + 1], axis=0))
                        nc.vector.tensor_tensor(
                            out=selT[:, :gn * P].rearrange("p (j q) -> p j q", j=gn),
                            in0=dloc_sb[:, b * nch + c0:b * nch + c0 + gn]
                                .unsqueeze(-1).to_broadcast([P, gn, P]),
                            in1=iota_row[:].unsqueeze(1).to_broadcast([P, gn, P]),
                            op=mybir.AluOpType.is_equal)
                        for j in range(gn):
                            sel_ps = pp.tile([P, P], f32, name=f"sel_ps{l}", tag="sel_ps", bufs=2)
                            nc.tensor.transpose(out=sel_ps[:],
                                                in_=selT[:, j * P:(j + 1) * P],
                                                identity=ident[:])
                            sel_sb = sb.tile([P, P], f32, name=f"sel_sb{l}", tag="sel_sb", bufs=4)
                            nc.vector.tensor_copy(sel_sb[:], sel_ps[:])
                            ad_ps = pp.tile([P, nh], f32, name=f"ad_ps{l}", tag="ad_ps", bufs=2)
                            nc.tensor.matmul(out=ad_ps[:], lhsT=sel_sb[:],
                                             rhs=ad_sb[l][:, b * nh:(b + 1) * nh],
                                             start=True, stop=True)
                            nc.vector.tensor_tensor(
                                out=u[:, j * nh:(j + 1) * nh],
                                in0=g[:, j * wl + hw:j * wl + hw + nh],
                                in1=ad_ps[:], op=mybir.AluOpType.add)
                        nc.vector.tensor_scalar_mul(un[:, :gn * nh], u[:, :gn * nh], 0.2)
                        nc.vector.tensor_tensor(out=u[:, :gn * nh], in0=u[:, :gn * nh],
                                                in1=un[:, :gn * nh],
                                                op=mybir.AluOpType.max)
                        nc.scalar.activation(ee[:, :gn * nh], u[:, :gn * nh],
                                             mybir.ActivationFunctionType.Exp)
                        nc.vector.tensor_tensor(
                            out=m[:, :gn * hw].rearrange("p (j h c) -> p j h c", j=gn, h=nh),
                            in0=g[:, :gn * wl].rearrange("p (j w) -> p j w", j=gn)[:, :, :hw]
                                .rearrange("p j (h c) -> p j h c", h=nh),
                            in1=ee[:, :gn * nh].rearrange("p (j h) -> p j h", j=gn)
                                .unsqueeze(-1).to_broadcast([P, gn, nh, ch]),
                            op=mybir.AluOpType.mult)
                        for j in range(gn):
                            c = c0 + j
                            nc.tensor.matmul(out=ps_out[:],
                                             lhsT=selT[:, j * P:(j + 1) * P],
                                             rhs=m[:, j * hw:(j + 1) * hw],
                                             start=(c == 0), stop=(c == nch - 1),
                                             skip_group_check=True)
                            nc.tensor.matmul(out=ps_s[:],
                                             lhsT=selT[:, j * P:(j + 1) * P],
                                             rhs=ee[:, j * nh:(j + 1) * nh],
                                             start=(c == 0), stop=(c == nch - 1),
                                             skip_group_check=True)
                    # ---- block finalize ----
                    rs = sb.tile([P, nh], f32, name=f"rs{l}", tag="rs", bufs=2)
                    nc.vector.reciprocal(rs[:], ps_s[:])
                    ob = sb.tile([P, hw], f32, name=f"ob{l}", tag="ob", bufs=2)
                    nc.vector.tensor_tensor(
                        out=ob[:].rearrange("p (h c) -> p h c", h=nh),
                        in0=ps_out[:].rearrange("p (h c) -> p h c", h=nh),
                        in1=rs[:].unsqueeze(-1).to_broadcast([P, nh, ch]),
                        op=mybir.AluOpType.mult)
                    if l < 2:
                        # BN (scale/shift per column) + ELU
                        nc.vector.tensor_tensor(
                            out=ob[:], in0=ob[:],
                            in1=bn_sb[l][:, :HC],
                            op=mybir.AluOpType.mult)
                        nc.vector.tensor_tensor(
                            out=ob[:], in0=ob[:],
                            in1=bn_sb[l][:, HC:],
                            op=mybir.AluOpType.add)
                        tneg = sb.tile([P, hw], f32, name=f"tneg{l}", tag="tneg", bufs=2)
                        nc.vector.tensor_scalar_min(tneg[:], ob[:], 0.0)
                        nc.scalar.activation(tneg[:], tneg[:],
                                             mybir.ActivationFunctionType.Exp)
                        # tneg = 0.2*exp(min(y,0)) - 0.2
                        nc.vector.tensor_scalar(
                            out=tneg[:], in0=tneg[:], scalar1=0.2, scalar2=-0.2,
                            op0=mybir.AluOpType.mult, op1=mybir.AluOpType.add)
                        nc.vector.tensor_scalar_max(ob[:], ob[:], 0.0)
                        nc.vector.tensor_tensor(out=ob[:], in0=ob[:], in1=tneg[:],
                                                op=mybir.AluOpType.add)
                        # next layer rows: eluT @ W_{l+1}ext
                        eT_ps = pp.tile([P, P], f32, name=f"eT_ps{l}", tag="misc_ps", bufs=1)
                        nc.tensor.transpose(out=eT_ps[:], in_=ob[:], identity=ident[:])
                        eT_sb = sb.tile([P, P], f32, name=f"eT_sb{l}", tag="eT_sb", bufs=2)
                        nc.vector.tensor_copy(eT_sb[:], eT_ps[:])
                        wnext = w1_sb if l == 0 else w2_sb
                        wn = W1T + H if l == 0 else W2T + 1
                        nhn = H if l == 0 else 1
                        row_ps = pp.tile([P, wn], f32, name=f"row_ps{l}", tag="misc_ps", bufs=1)
                        nc.tensor.matmul(out=row_ps[:], lhsT=eT_sb[:],
                                         rhs=wnext[:, :wn], start=True, stop=True)
                        row_sb2 = sb.tile([P, wn], f32, name=f"row_sb2{l}", tag="row_sb2", bufs=2)
                        nc.vector.tensor_copy(row_sb2[:], row_ps[:])
                        nc.sync.dma_start(shard[l + 1][b * P:(b + 1) * P, :],
                                          row_sb2[:, :wn - nhn])
                        if l == 0:
                            nc.sync.dma_start(dbg1_t[b * P:(b + 1) * P, :],
                                              row_sb2[:, :W1T])
                        nc.vector.tensor_copy(
                            ad_sb[l + 1][:, b * nhn:(b + 1) * nhn],
                            row_sb2[:, wn - nhn:wn])
                    else:
                        nc.vector.tensor_tensor(
                            out=ob[:], in0=ob[:],
                            in1=b2_sb[:],
                            op=mybir.AluOpType.add)
                        nc.sync.dma_start(out_t[b * P:(b + 1) * P, :], ob[:])
                if l < 2:
                    nc.gpsimd.collective_compute(
                        "AllGather", mybir.AluOpType.bypass,
                        replica_groups=[list(range(NCORE))],
                        ins=[shard[l + 1][:]], outs=[tabs[l + 1][0:NTAB, :]])
    nc.compile()
    return nc


def _host_inputs(inputs, offs, dloc):
    x = np.asarray(inputs['x'], np.float32)
    xpad = np.zeros((NCORE * ROWS_PER_CORE, F_IN), np.float32)
    xpad[:N] = x

    def bd(a):  # [H, C] -> block-diag [H*C, H]
        hh, cc = a.shape
        m = np.zeros((hh * cc, hh), np.float32)
        for h in range(hh):
            m[h * cc:(h + 1) * cc, h] = a[h]
        return m

    W0, W1, W2 = [np.asarray(inputs[k], np.float32) for k in ('W0', 'W1', 'W2')]
    w0ext = np.concatenate([W0, W0 @ bd(np.asarray(inputs['as0'])),
                            W0 @ bd(np.asarray(inputs['ad0']))], axis=1)
    w1ext = np.concatenate([W1, W1 @ bd(np.asarray(inputs['as1'])),
                            W1 @ bd(np.asarray(inputs['ad1']))], axis=1)
    w2ext = np.concatenate([W2, W2 @ bd(np.asarray(inputs['as2'])),
                            W2 @ bd(np.asarray(inputs['ad2']))], axis=1)

    def bnfold(g, be, rm, rv, b):
        sc = np.asarray(g) / np.sqrt(np.asarray(rv) + BN_EPS)
        sh = (np.asarray(b) - np.asarray(rm)) * sc + np.asarray(be)
        return np.concatenate([np.tile(sc[None, :], (P, 1)), np.tile(sh[None, :], (P, 1))]).astype(np.float32)

    bn0 = bnfold(inputs['g0'], inputs['be0'], inputs['rm0'], inputs['rv0'], inputs['b0'])
    bn1 = bnfold(inputs['g1'], inputs['be1'], inputs['rm1'], inputs['rv1'], inputs['b1'])
    b2 = np.tile(np.asarray(inputs['b2'], np.float32)[None, :], (P, 1))

    in_maps = []
    for c in range(NCORE):
        xT = xpad[c * ROWS_PER_CORE:(c + 1) * ROWS_PER_CORE].T.copy()
        in_maps.append({
            'xT': xT, 'offs': offs[c], 'dloc': dloc[c],
            'w0ext': w0ext, 'w1ext': w1ext, 'w2ext': w2ext,
            'bn0': bn0, 'bn1': bn1, 'b2': b2,
        })
    return in_maps


def kernel(**inputs):
    edge_index = np.asarray(inputs['edge_index'])
    offs, dloc, nch = _preprocess(edge_index)
    if nch not in _CACHE:
        _CACHE[nch] = _build_program(nch)
    nc = _CACHE[nch]
    in_maps = _host_inputs(inputs, offs, dloc)
    res = bass_utils.run_bass_kernel_spmd(nc, in_maps, core_ids=list(range(NCORE)))
    out = np.concatenate([res.results[c]['out'] for c in range(NCORE)], axis=0)
    return out[:N].astype(np.float32)
